# revision 15
# baseline (speedup 1.0000x reference)
"""HEAGNN Trainium2 kernel: 4-layer edge-gated GNN + per-graph attention.

Sharding: graphs/nodes/edges-by-dst split across 8 cores. Per layer each core
computes node projections for its 2048-node block, AllGathers the row-major
[m | h@gw_j] gather table, gathers per-edge rows with dma_gather, applies the
edge gate, scatter-adds messages via windowed one-hot matmuls accumulating in
PSUM, then runs dense per-graph attention on its 4 graphs. Host pre-folds all
LayerNorm gains/means that feed linear layers and builds the sorted/padded
edge schedule.
"""

import math
import os

import ml_dtypes
import numpy as np

import concourse.bacc as bacc
import concourse.bass as bass
import concourse.mybir as mybir
import concourse.tile as tile
from concourse.bass_utils import run_bass_kernel_spmd

F32 = mybir.dt.float32
BF16 = mybir.dt.bfloat16
I16 = mybir.dt.int16
AF = mybir.ActivationFunctionType
ALU = mybir.AluOpType

P = 128
EPS = 1e-5

# Full-problem dims
DIMS_FULL = dict(NN=16384, E=262144, B=32, H=128, L=4, NH=4, C=8)

LAST_RESULT = None  # stashed BassKernelResults for test harness


# ----------------------------------------------------------------------------
# Host-side math helpers
# ----------------------------------------------------------------------------

def _bf(x):
    return np.ascontiguousarray(np.asarray(x, np.float32).astype(ml_dtypes.bfloat16))


def _f32(x):
    return np.ascontiguousarray(np.asarray(x, np.float32))


def _ln_fold_pre(Wm, b, g):
    """Fold LN that FOLLOWS z=u@Wm+b.  Returns (Wfull [(in+1),H], rstd_fn).
    device computes act(aug @ Wfull + beta) with aug=[u*rstd, rstd]."""
    Wm = np.asarray(Wm, np.float64)
    b = np.asarray(b, np.float64)
    g = np.asarray(g, np.float64)
    H = Wm.shape[1]
    Wc = Wm - Wm.mean(axis=1, keepdims=True)
    bc = b - b.mean()
    A = (Wc @ Wc.T) / H
    cvec = (Wc @ bc) / H
    dconst = float((bc * bc).mean())

    def rstd_fn(u):
        u = np.asarray(u, np.float64)
        q = np.einsum("ni,ij,nj->n", u, A, u) + 2.0 * (u @ cvec) + dconst
        return 1.0 / np.sqrt(q + EPS)

    Wg = Wc * g[None, :]
    bg = bc * g
    Wfull = np.concatenate([Wg, bg[None, :]], axis=0)  # [(in+1), H]
    return Wfull, rstd_fn


def _aug(u, rstd):
    """[n,k],[n] -> [k+1, n] feature-major augmented input."""
    a = np.concatenate([u * rstd[:, None], rstd[:, None]], axis=1)
    return np.ascontiguousarray(a.T)


def _wrap16(idx):
    """Wrap an index list (len % 16 == 0) into dma_gather layout [16, n/16]."""
    return np.ascontiguousarray(idx.reshape(-1, 16).T)


# ----------------------------------------------------------------------------
# Host preprocessing: shard + sort + pad edges, fold weights
# ----------------------------------------------------------------------------

def preprocess(inputs, D):
    NN, E, B, H, L, NH, C = (D[k] for k in ("NN", "E", "B", "H", "L", "NH", "C"))
    NB = NN // C          # nodes per core
    GB = B // C           # graphs per core
    NG = NN // B          # nodes per graph
    W = NB // P           # 128-node windows per core

    src = np.asarray(inputs["edge_index"][0], np.int64)
    dst = np.asarray(inputs["edge_index"][1], np.int64)
    ea = _f32(inputs["edge_attr"])

    order = np.argsort(dst, kind="stable")
    src_s, dst_s, ea_s = src[order], dst[order], ea[order]

    win_of = dst_s // P  # global window id 0..NN/P
    # counts per global window
    counts = np.bincount(win_of, minlength=NN // P)
    T = int(math.ceil(counts.max() / P))
    T = max(T, 1)
    NT = W * T
    EPAD = NT * P

    # per-core padded arrays
    src_pad = np.zeros((C, EPAD), np.int64)
    dstloc_pad = np.zeros((C, EPAD), np.int64)
    dwin_pad = np.full((C, EPAD), 200.0, np.float32)  # sentinel: no one-hot match
    ea_pad = np.zeros((C, EPAD, ea.shape[1]), np.float32)
    valid_pad = np.zeros((C, EPAD), bool)

    wstart = np.concatenate([[0], np.cumsum(counts)])
    for c in range(C):
        for w in range(W):
            gw = c * W + w
            lo, hi = wstart[gw], wstart[gw + 1]
            n = hi - lo
            off = w * T * P
            src_pad[c, off:off + n] = src_s[lo:hi]
            dstloc_pad[c, off:off + n] = dst_s[lo:hi] - c * NB
            dwin_pad[c, off:off + n] = (dst_s[lo:hi] - c * NB - w * P).astype(np.float32)
            ea_pad[c, off:off + n] = ea_s[lo:hi]
            valid_pad[c, off:off + n] = True

    # LN folds -------------------------------------------------------------
    W6x, rstd_x = _ln_fold_pre(inputs["ne_w"], inputs["ne_b"], inputs["ne_g"])
    W4e, rstd_e = _ln_fold_pre(inputs["ee_w"], inputs["ee_b"], inputs["ee_g"])
    W9a, rstd_a = _ln_fold_pre(inputs["am_w1"], inputs["am_b1"], inputs["am_g"])

    x = _f32(inputs["x"])
    augx = np.stack([
        _aug(x[c * NB:(c + 1) * NB], rstd_x(x[c * NB:(c + 1) * NB])) for c in range(C)
    ])  # [C, 6, NB]
    auge = np.stack([
        _aug(ea_pad[c], rstd_e(ea_pad[c])) for c in range(C)
    ])  # [C, 4, EPAD]
    af = _f32(inputs["additional_features"])
    augaf = np.stack([
        _aug(af[c * GB:(c + 1) * GB], rstd_a(af[c * GB:(c + 1) * GB])) for c in range(C)
    ])  # [C, 9, GB]

    # wrapped int16 gather indices, per window, tiled to 128 partitions
    def wrapped(idx2d):  # [C, EPAD] -> [C, 128, NT*8]
        outs = []
        for c in range(C):
            cols = [
                _wrap16(idx2d[c, w * T * P:(w + 1) * T * P].astype(np.int16))
                for w in range(W)
            ]
            m = np.concatenate(cols, axis=1)  # [16, NT*8]
            outs.append(np.tile(m, (8, 1)))
        return np.stack(outs)

    srcw = wrapped(src_pad)
    dstw = wrapped(dstloc_pad)

    # dwin per tile: [C, 128, NT]  column t = tile t's 128 values
    dwin = np.stack([
        np.ascontiguousarray(dwin_pad[c].reshape(NT, P).T) for c in range(C)
    ])

    # attention folds per layer --------------------------------------------
    at_g = _f32(inputs["at_g"])
    at_b = _f32(inputs["at_beta"])
    qwf, kwf, vwf, owf = [], [], [], []
    qbf, kbf, obf = [], [], []
    for l in range(L):
        qw = _f32(inputs["qw"][l]); kw = _f32(inputs["kw"][l])
        vw = _f32(inputs["vw"][l]); ow = _f32(inputs["ow_"][l])
        qwf.append(at_g[l][:, None] * qw)
        kwf.append(at_g[l][:, None] * kw)
        vwf.append(at_g[l][:, None] * vw)
        owf.append(ow)
        qbf.append(at_b[l] @ qw + _f32(inputs["qb"][l]))
        kbf.append(at_b[l] @ kw + _f32(inputs["kb"][l]))
        vb_ = at_b[l] @ vw + _f32(inputs["vb"][l])
        obf.append(vb_ @ ow + _f32(inputs["ob_"][l]))

    # weight blob (bf16 [128, 128] tiles, order matters: 4x gwe first)
    HB = H  # tiles are [H, H]
    blob = []
    blob_idx = {}

    def addw(name, w):
        w = _f32(w)
        assert w.shape == (HB, HB), (name, w.shape)
        blob_idx[name] = len(blob)
        blob.append(w)

    def addw2(name, w):
        w = _f32(w)
        hi = _bf(w).astype(np.float32)
        addw(name, hi)
        addw(name + "~lo", w - hi)

    gw_all = _f32(inputs["gw"])  # [L, 3H, H]
    for l in range(L):
        addw(f"gwe{l}", gw_all[l][:H])
    for l in range(L):
        addw(f"gwi{l}", gw_all[l][H:2 * H])
        addw(f"gwj{l}", gw_all[l][2 * H:3 * H])
        addw2(f"w1_{l}", inputs["nm_w1"][l])
        addw2(f"w2_{l}", inputs["nm_w2"][l])
        addw(f"qw{l}", qwf[l])
        addw(f"kw{l}", kwf[l])
        addw2(f"vw{l}", vwf[l])
        addw2(f"ow{l}", owf[l])
    ow1 = _f32(inputs["o_w1"])  # [4H, 2H]
    for kc in range(4):
        for mc in range(2):
            addw2(f"ow1_{kc}_{mc}", ow1[kc * H:(kc + 1) * H, mc * H:(mc + 1) * H])
    ow2 = _f32(inputs["o_w2"])  # [2H, H]
    for kc in range(2):
        addw2(f"ow2_{kc}", ow2[kc * H:(kc + 1) * H])
    addw2("amw2", inputs["am_w2"])
    wblob = _bf(np.concatenate(blob, axis=1))  # [128, 128*nw]

    # bias/vector blob f32 [128, nv]
    vecs = []
    vec_idx = {}

    def addv(name, v, n=None):
        v = _f32(v).reshape(-1)
        col = np.zeros((P,), np.float32)
        col[: v.shape[0]] = v
        vec_idx[name] = len(vecs)
        vecs.append(col)

    for l in range(L):
        addv(f"b1_{l}", inputs["nm_b1"][l])
        addv(f"g_{l}", inputs["nm_g"][l])
        addv(f"gK_{l}", _f32(inputs["nm_g"][l]) / H)
        addv(f"beta_{l}", inputs["nm_beta"][l])
        addv(f"b2_{l}", inputs["nm_b2"][l])
        addv(f"gb_{l}", inputs["gb"][l])
        addv(f"qb_{l}", qbf[l])
        addv(f"kb_{l}", kbf[l])
        addv(f"ob_{l}", obf[l])
    addv("nebeta", inputs["ne_beta"])
    addv("eebeta", inputs["ee_beta"])
    addv("ambeta", inputs["am_beta"])
    addv("amb2", inputs["am_b2"])
    og1 = _f32(inputs["o_g1"]); ob1 = _f32(inputs["o_beta1"]); obi1 = _f32(inputs["o_b1"])
    addv("og1a", og1[:H]); addv("og1b", og1[H:])
    addv("og1aK", og1[:H] / (2 * H)); addv("og1bK", og1[H:] / (2 * H))
    addv("obeta1a", ob1[:H]); addv("obeta1b", ob1[H:])
    addv("ob1a", obi1[:H]); addv("ob1b", obi1[H:])
    addv("og2", inputs["o_g2"]); addv("og2K", _f32(inputs["o_g2"]) / H)
    addv("obeta2", inputs["o_beta2"]); addv("ob2", inputs["o_b2"])
    addv("ob3", inputs["o_b3"])
    addv("ob4", inputs["o_b4"])
    addv("eps", np.full((P,), EPS, np.float32))
    vblob = _f32(np.stack(vecs, axis=1))  # [128, nv]

    # small-K weights
    def hilo(w):
        w = _f32(w)
        hi = _bf(w)
        return hi, _bf(w - hi.astype(np.float32))

    w6x_h, w6x_l = hilo(W6x)
    w4e_h, w4e_l = hilo(W4e)
    w9a_h, w9a_l = hilo(W9a)
    ow3_h, ow3_l = hilo(inputs["o_w3"])
    ow4_h, ow4_l = hilo(inputs["o_w4"])
    smalls = dict(
        w6x=w6x_h, w6x_lo=w6x_l,
        w4e=w4e_h, w4e_lo=w4e_l,
        w9a=w9a_h, w9a_lo=w9a_l,
        ow3=ow3_h, ow3_lo=ow3_l,
        ow4=ow4_h, ow4_lo=ow4_l,
    )

    # constants
    iota_r = _bf(np.tile(np.arange(P, dtype=np.float32)[None, :], (P, 1)))
    identy = _bf(np.eye(P, dtype=np.float32))
    ones_b = _bf(np.ones((P, P), np.float32))
    shead = _bf((np.arange(P)[None, :] // (H // NH) == np.arange(NH)[:, None]))

    meta = dict(NB=NB, GB=GB, NG=NG, W=W, T=T, NT=NT, EPAD=EPAD,
                blob_idx=blob_idx, vec_idx=vec_idx, nw=len(blob), nv=len(vecs))

    per_core = []
    for c in range(C):
        per_core.append(dict(
            augx=_bf(augx[c]), auge=_bf(auge[c]), augaf=_bf(augaf[c]),
            srcw=srcw[c].astype(np.int16), dstw=dstw[c].astype(np.int16),
            dwin=_bf(dwin[c]),
            wblob=wblob, vblob=vblob,
            iota_r=iota_r, identy=identy, ones_b=ones_b, shead=shead,
            **smalls,
        ))
    return per_core, meta


# ----------------------------------------------------------------------------
# Device program
# ----------------------------------------------------------------------------

class Ctx:
    pass


def _ln_fm(g, zbs, n, Kfeat, outs, gain=None, gainK=None, beta=None, relu=False,
           s_keep=None):
    """Feature-major LayerNorm over partition dim (Kfeat = 128 or 256).
    zbs: list of bf16 SBUF APs [128, n] (feature tiles).  outs: same-shaped
    bf16 out APs.  gain/gainK/beta: [128,1] f32 APs per tile (or None)."""
    nc, ps, tp = g.nc, g.psum, g.tmp
    s_ps = ps.tile([P, n], F32, tag="b512", bufs=2)
    for i, z in enumerate(zbs):
        nc.tensor.matmul(s_ps[:], g.ones_b[:], z, start=(i == 0), stop=(i == len(zbs) - 1))
    s_sb = tp.tile([P, n], F32, tag="ln_ssb", bufs=2)
    nc.vector.tensor_copy(out=s_sb[:], in_=s_ps[:])
    q_ps = ps.tile([P, n], F32, tag="b512", bufs=2)
    for i, z in enumerate(zbs):
        z2 = tp.tile([P, n], BF16, tag="ln_z2", bufs=2)
        nc.vector.tensor_tensor(out=z2[:], in0=z, in1=z, op=ALU.mult)
        nc.tensor.matmul(q_ps[:], g.ones_b[:], z2[:], start=(i == 0), stop=(i == len(zbs) - 1))
    t1 = tp.tile([P, n], F32, tag="ln_x", bufs=2)
    nc.vector.scalar_tensor_tensor(out=t1[:], in0=s_sb[:], scalar=1.0 / Kfeat,
                                   in1=s_sb[:], op0=ALU.mult, op1=ALU.mult)
    d = tp.tile([P, n], F32, tag="ln_y", bufs=2)
    nc.vector.tensor_tensor(out=d[:], in0=q_ps[:], in1=t1[:], op=ALU.subtract)
    sd = tp.tile([P, n], F32, tag="ln_z", bufs=2)
    nc.scalar.activation(sd[:], d[:], AF.Sqrt, bias=g.eps, scale=1.0 / Kfeat)
    rstd = tp.tile([P, n], F32, tag="ln_rs", bufs=2)
    nc.vector.reciprocal(rstd[:], sd[:])
    for i, (z, out) in enumerate(zip(zbs, outs)):
        a1 = tp.tile([P, n], F32, tag="ln_x", bufs=2)
        nc.vector.scalar_tensor_tensor(out=a1[:], in0=z, scalar=(gain[i] if gain else 1.0),
                                       in1=rstd[:], op0=ALU.mult, op1=ALU.mult)
        a2 = tp.tile([P, n], F32, tag="ln_y", bufs=2)
        nc.vector.scalar_tensor_tensor(out=a2[:], in0=s_sb[:],
                                       scalar=(gainK[i] if gainK else 1.0 / Kfeat),
                                       in1=rstd[:], op0=ALU.mult, op1=ALU.mult)
        a3 = tp.tile([P, n], F32, tag="ln_z", bufs=2)
        nc.vector.tensor_tensor(out=a3[:], in0=a1[:], in1=a2[:], op=ALU.subtract)
        nc.scalar.activation(out, a3[:], AF.Relu if relu else AF.Identity,
                             bias=(beta[i] if beta else 0.0))


def build_program(D, meta, num_cores):
    NN, E, B, H, L, NH, C = (D[k] for k in ("NN", "E", "B", "H", "L", "NH", "C"))
    NB, GB, NG, W, T, NT, EPAD = (meta[k] for k in ("NB", "GB", "NG", "W", "T", "NT", "EPAD"))
    HD = H // NH
    nchunk = NB // 512 if NB >= 512 else 1
    CH = min(512, NB)  # node-phase chunk width
    assert NB % CH == 0

    nc = bacc.Bacc("TRN2", target_bir_lowering=False, debug=False,
                   num_devices=num_cores)

    def din(name, shape, dt):
        return nc.dram_tensor(name, list(shape), dt, kind="ExternalInput").ap()

    ins = dict(
        augx=din("augx", (6, NB), BF16),
        auge=din("auge", (4, EPAD), BF16),
        augaf=din("augaf", (9, GB), BF16),
        srcw=din("srcw", (P, NT * 8), I16),
        dstw=din("dstw", (P, NT * 8), I16),
        dwin=din("dwin", (P, NT), BF16),
        wblob=din("wblob", (P, P * meta["nw"]), BF16),
        vblob=din("vblob", (P, meta["nv"]), F32),
        iota_r=din("iota_r", (P, P), BF16),
        identy=din("identy", (P, P), BF16),
        ones_b=din("ones_b", (P, P), BF16),
        shead=din("shead", (NH, P), BF16),
        w6x=din("w6x", (6, H), BF16),
        w4e=din("w4e", (4, H), BF16),
        w9a=din("w9a", (9, H), BF16),
        ow3=din("ow3", (H, 64), BF16),
        ow4=din("ow4", (64, 1), BF16),
        w6x_lo=din("w6x_lo", (6, H), BF16),
        w4e_lo=din("w4e_lo", (4, H), BF16),
        w9a_lo=din("w9a_lo", (9, H), BF16),
        ow3_lo=din("ow3_lo", (H, 64), BF16),
        ow4_lo=din("ow4_lo", (64, 1), BF16),
    )
    out_dram = nc.dram_tensor("out", [1, GB], F32, kind="ExternalOutput").ap()

    dbg = {}
    if bool(int(os.environ.get("BASS_DEBUG_DUMPS", "0"))):
        def dout(name, shape, dt=F32):
            dbg[name] = nc.dram_tensor(name, list(shape), dt, kind="ExternalOutput").ap()
        dout("dbg_h0", (P, NB))
        dout("dbg_ea", (P, EPAD), BF16)
        for l in range(L):
            dout(f"dbg_m{l}", (P, NB))
            dout(f"dbg_hconv{l}", (P, NB))
            dout(f"dbg_hattn{l}", (P, NB))
        for nm in ("mean4", "max4", "addt", "t1o0", "t1o1", "t1n0", "t1n1",
                   "t2b", "t2n"):
            dout(f"dbg_{nm}", (P, GB), BF16)
        dout("dbg_t3b", (64, GB), BF16)

    ea_pm = nc.dram_tensor("ea_pm", [P, EPAD], BF16).ap()
    gi_dram = nc.dram_tensor("gi_dram", [NB, H], BF16).ap()
    mg_in = nc.dram_tensor("mg_in", [NB, 2 * H], BF16).ap()
    mg_tab = nc.dram_tensor("mg_tab", [NN, 2 * H], BF16, addr_space="Shared").ap()

    bi, vi = meta["blob_idx"], meta["vec_idx"]

    with tile.TileContext(nc) as tc:
        g = Ctx()
        g.nc = nc
        import contextlib
        stack = contextlib.ExitStack()
        cpool = stack.enter_context(tc.tile_pool(name="const", bufs=1))
        g.psum = stack.enter_context(tc.tile_pool(name="ps", bufs=1, space="PSUM"))
        g.tmp = stack.enter_context(tc.tile_pool(name="tmp", bufs=3))
        pers = stack.enter_context(tc.tile_pool(name="pers", bufs=1))
        ep = stack.enter_context(tc.tile_pool(name="ep", bufs=2))
        et = stack.enter_context(tc.tile_pool(name="et", bufs=4))
        ap_ = stack.enter_context(tc.tile_pool(name="ap", bufs=2))

        # ---- load constants ----
        wblob_sb = cpool.tile([P, P * meta["nw"]], BF16)
        nc.sync.dma_start(wblob_sb[:], ins["wblob"])
        vblob_sb = cpool.tile([P, meta["nv"]], F32)
        nc.sync.dma_start(vblob_sb[:], ins["vblob"])

        def wt(name):
            i = bi[name]
            return wblob_sb[:, i * P:(i + 1) * P]

        def vc(name):
            i = vi[name]
            return vblob_sb[:, i:i + 1]

        g.eps = vc("eps")
        g.ones_b = cpool.tile([P, P], BF16)
        nc.sync.dma_start(g.ones_b[:], ins["ones_b"])
        iota_sb = cpool.tile([P, P], BF16)
        nc.sync.dma_start(iota_sb[:], ins["iota_r"])
        ident_sb = cpool.tile([P, P], BF16)
        nc.sync.dma_start(ident_sb[:], ins["identy"])
        identf_sb = cpool.tile([P, P], F32)
        nc.vector.tensor_copy(out=identf_sb[:], in_=ident_sb[:])
        shead_sb = cpool.tile([NH, P], BF16)
        nc.sync.dma_start(shead_sb[:], ins["shead"])
        w6x_sb = cpool.tile([6, H], BF16)
        nc.sync.dma_start(w6x_sb[:], ins["w6x"])
        w4e_sb = cpool.tile([4, H], BF16)
        nc.sync.dma_start(w4e_sb[:], ins["w4e"])
        w9a_sb = cpool.tile([9, H], BF16)
        nc.sync.dma_start(w9a_sb[:], ins["w9a"])
        ow3_sb = cpool.tile([H, 64], BF16)
        nc.sync.dma_start(ow3_sb[:], ins["ow3"])
        ow4_sb = cpool.tile([64, 1], BF16)
        nc.sync.dma_start(ow4_sb[:], ins["ow4"])
        w6xl_sb = cpool.tile([6, H], BF16)
        nc.sync.dma_start(w6xl_sb[:], ins["w6x_lo"])
        w4el_sb = cpool.tile([4, H], BF16)
        nc.sync.dma_start(w4el_sb[:], ins["w4e_lo"])
        w9al_sb = cpool.tile([9, H], BF16)
        nc.sync.dma_start(w9al_sb[:], ins["w9a_lo"])
        ow3l_sb = cpool.tile([H, 64], BF16)
        nc.sync.dma_start(ow3l_sb[:], ins["ow3_lo"])
        ow4l_sb = cpool.tile([64, 1], BF16)
        nc.sync.dma_start(ow4l_sb[:], ins["ow4_lo"])

        def mm2(psum_ap, name, rhs):
            nc.tensor.matmul(psum_ap, wt(name), rhs, start=True, stop=False)
            nc.tensor.matmul(psum_ap, wt(name + "~lo"), rhs, start=False, stop=True)

        srcw_sb = pers.tile([P, NT * 8], I16)
        nc.sync.dma_start(srcw_sb[:], ins["srcw"])
        dstw_sb = pers.tile([P, NT * 8], I16)
        nc.sync.dma_start(dstw_sb[:], ins["dstw"])
        dwin_sb = pers.tile([P, NT], BF16)
        nc.sync.dma_start(dwin_sb[:], ins["dwin"])

        h_sb = pers.tile([P, NB], F32)      # node features, feature-major (f32 residual)
        m_sb = pers.tile([P, NB], F32)      # node_mlp output (per layer)
        gj_sb = pers.tile([P, NB], BF16)
        gi_sb = pers.tile([P, NB], BF16)

        # ---- embed: h0 ----
        augx_sb = cpool.tile([6, NB], BF16)
        nc.sync.dma_start(augx_sb[:], ins["augx"])
        for k in range(NB // CH):
            sl = slice(k * CH, (k + 1) * CH)
            zp = g.psum.tile([P, CH], F32, tag="b512", bufs=2)
            nc.tensor.matmul(zp[:], w6x_sb[:], augx_sb[:, sl], start=True, stop=False)
            nc.tensor.matmul(zp[:], w6xl_sb[:], augx_sb[:, sl], start=False, stop=True)
            nc.scalar.activation(h_sb[:, sl], zp[:], AF.Relu, bias=vc("nebeta"))

        # ---- embed: ea -> ea_pm (feature-major, bf16) ----
        ECH = 512
        for off in range(0, EPAD, ECH):
            n = min(ECH, EPAD - off)
            sl = slice(off, off + n)
            ae = g.tmp.tile([4, ECH], BF16, tag="auge")
            nc.sync.dma_start(ae[:, :n], ins["auge"][:, sl])
            zp = g.psum.tile([P, ECH], F32, tag="b512", bufs=2)
            nc.tensor.matmul(zp[:, :n], w4e_sb[:], ae[:, :n], start=True, stop=False)
            nc.tensor.matmul(zp[:, :n], w4el_sb[:], ae[:, :n], start=False, stop=True)
            eat = g.tmp.tile([P, ECH], BF16, tag="eat")
            nc.scalar.activation(eat[:, :n], zp[:, :n], AF.Relu, bias=vc("eebeta"))
            nc.sync.dma_start(ea_pm[:, sl], eat[:, :n])
            if "dbg_ea" in dbg:
                nc.sync.dma_start(dbg["dbg_ea"][:, sl], eat[:, :n])

        if "dbg_h0" in dbg:
            nc.sync.dma_start(dbg["dbg_h0"][:], h_sb[:])

        # ---- layers ----
        for l in range(L):
            # node phase: m, gj, gi on own block
            for k in range(NB // CH):
                sl = slice(k * CH, (k + 1) * CH)
                hb = g.tmp.tile([P, CH], BF16, tag="hb")
                nc.vector.tensor_copy(out=hb[:], in_=h_sb[:, sl])
                zp = g.psum.tile([P, CH], F32, tag="b512", bufs=2)
                mm2(zp[:], f"w1_{l}", hb[:])
                zb = g.tmp.tile([P, CH], BF16, tag="zb")
                nc.scalar.activation(zb[:], zp[:], AF.Identity, bias=vc(f"b1_{l}"))
                tb = g.tmp.tile([P, CH], BF16, tag="tb")
                _ln_fm(g, [zb[:]], CH, H, [tb[:]], gain=[vc(f"g_{l}")],
                       gainK=[vc(f"gK_{l}")], beta=[vc(f"beta_{l}")], relu=True)
                mp = g.psum.tile([P, CH], F32, tag="b512", bufs=2)
                mm2(mp[:], f"w2_{l}", tb[:])
                nc.scalar.activation(m_sb[:, sl], mp[:], AF.Identity, bias=vc(f"b2_{l}"))
                gp = g.psum.tile([P, CH], F32, tag="b512", bufs=2)
                nc.tensor.matmul(gp[:], wt(f"gwj{l}"), hb[:], start=True, stop=True)
                nc.scalar.activation(gj_sb[:, sl], gp[:], AF.Identity)
                gp2 = g.psum.tile([P, CH], F32, tag="b512", bufs=2)
                nc.tensor.matmul(gp2[:], wt(f"gwi{l}"), hb[:], start=True, stop=True)
                nc.scalar.activation(gi_sb[:, sl], gp2[:], AF.Identity, bias=vc(f"gb_{l}"))

            # transpose m|gj -> mg_in rows; gi -> gi_dram rows
            for i in range(NB // P):
                tsl = slice(i * P, (i + 1) * P)
                stg = g.tmp.tile([P, 2 * H], BF16, tag="mgstg")
                tp1 = g.psum.tile([P, P], F32, tag="b128", bufs=2)
                nc.tensor.transpose(tp1[:], m_sb[:, tsl], identf_sb[:])
                nc.vector.tensor_copy(out=stg[:, 0:H], in_=tp1[:])
                tp2 = g.psum.tile([P, P], BF16, tag="b128", bufs=2)
                nc.tensor.transpose(tp2[:], gj_sb[:, tsl], ident_sb[:])
                nc.vector.tensor_copy(out=stg[:, H:2 * H], in_=tp2[:])
                nc.sync.dma_start(mg_in[tsl, :], stg[:])
                tp3 = g.psum.tile([P, P], BF16, tag="b128", bufs=2)
                nc.tensor.transpose(tp3[:], gi_sb[:, tsl], ident_sb[:])
                gstg = g.tmp.tile([P, H], BF16, tag="gistg")
                nc.vector.tensor_copy(out=gstg[:], in_=tp3[:])
                nc.sync.dma_start(gi_dram[tsl, :], gstg[:])

            # AllGather mg table
            nc.gpsimd.collective_compute(
                "AllGather", ALU.bypass,
                replica_groups=[list(range(num_cores))],
                ins=[mg_in[:]], outs=[mg_tab[:]],
            )

            # edge phase
            for w in range(W):
                esl = slice(w * T * P, (w + 1) * T * P)
                isl = slice(w * T * 8, (w + 1) * T * 8)
                GCT = 4  # tiles per dma_gather call
                mgb = ep.tile([P, T * 2 * H], BF16, tag="mgb")
                gib = ep.tile([P, T * H], BF16, tag="gib")
                for t0 in range(0, T, GCT):
                    t1 = min(t0 + GCT, T)
                    nct = t1 - t0
                    i0 = w * T * 8 + t0 * 8
                    nc.gpsimd.dma_gather(
                        mgb[:, t0 * 2 * H:t1 * 2 * H].rearrange(
                            "p (t e) -> p t e", e=2 * H),
                        mg_tab[:], srcw_sb[:, i0:i0 + nct * 8],
                        nct * P, nct * P, 2 * H)
                    nc.gpsimd.dma_gather(
                        gib[:, t0 * H:t1 * H].rearrange("p (t e) -> p t e", e=H),
                        gi_dram[:], dstw_sb[:, i0:i0 + nct * 8],
                        nct * P, nct * P, H)
                eab = ep.tile([P, T * P], BF16, tag="eab")
                nc.sync.dma_start(eab[:], ea_pm[:, esl])

                agg = g.psum.tile([P, P], F32, tag="agg", bufs=1)
                for t in range(T):
                    ts = slice(t * P, (t + 1) * P)
                    eapp = g.psum.tile([P, P], F32, tag="b128", bufs=2)
                    nc.tensor.matmul(eapp[:], eab[:, ts], wt(f"gwe{l}"),
                                     start=True, stop=True)
                    sel = et.tile([P, P], BF16, tag="sel")
                    nc.vector.tensor_tensor(
                        out=sel[:], in0=dwin_sb[:, w * T + t:w * T + t + 1].to_broadcast([P, P]),
                        in1=iota_sb[:], op=ALU.is_equal)
                    t1 = et.tile([P, P], F32, tag="t1")
                    nc.vector.tensor_tensor(out=t1[:], in0=mgb[:, t * 2 * H + H:(t + 1) * 2 * H],
                                            in1=gib[:, t * H:(t + 1) * H], op=ALU.add)
                    t2 = et.tile([P, P], F32, tag="t2")
                    nc.vector.tensor_tensor(out=t2[:], in0=t1[:], in1=eapp[:], op=ALU.add)
                    gate = et.tile([P, P], BF16, tag="gate")
                    nc.scalar.activation(gate[:], t2[:], AF.Sigmoid)
                    msg = et.tile([P, P], BF16, tag="msg")
                    nc.vector.tensor_tensor(out=msg[:], in0=gate[:],
                                            in1=mgb[:, t * 2 * H:t * 2 * H + H], op=ALU.mult)
                    nc.tensor.matmul(agg[:], msg[:], sel[:],
                                     start=(t == 0), stop=(t == T - 1))

                hsl = slice(w * P, (w + 1) * P)
                cv = g.tmp.tile([P, P], F32, tag="cv")
                nc.vector.tensor_tensor(out=cv[:], in0=agg[:], in1=m_sb[:, hsl], op=ALU.add)
                nc.vector.tensor_tensor(out=h_sb[:, hsl], in0=h_sb[:, hsl], in1=cv[:], op=ALU.add)

            if f"dbg_hconv{l}" in dbg:
                nc.sync.dma_start(dbg[f"dbg_m{l}"][:], m_sb[:])
                nc.sync.dma_start(dbg[f"dbg_hconv{l}"][:], h_sb[:])

            # attention per graph
            for gr in range(GB):
                gsl = slice(gr * NG, (gr + 1) * NG)
                hgb = ap_.tile([P, NG], BF16, tag="hgb")
                nc.vector.tensor_copy(out=hgb[:], in_=h_sb[:, gsl])
                hn = ap_.tile([P, NG], BF16, tag="hn")
                _ln_fm(g, [hgb[:]], NG, H, [hn[:]])
                qp = g.psum.tile([P, NG], F32, tag="b512", bufs=2)
                nc.tensor.matmul(qp[:], wt(f"qw{l}"), hn[:], start=True, stop=True)
                qsb = ap_.tile([P, NG], BF16, tag="qsb")
                nc.scalar.activation(qsb[:], qp[:], AF.Identity, bias=vc(f"qb_{l}"))
                kp = g.psum.tile([P, NG], F32, tag="b512", bufs=2)
                nc.tensor.matmul(kp[:], wt(f"kw{l}"), hn[:], start=True, stop=True)
                ksb = ap_.tile([P, NG], BF16, tag="ksb")
                nc.scalar.activation(ksb[:], kp[:], AF.Identity, bias=vc(f"kb_{l}"))

                JT = NG // P
                vaugs = []
                for jt in range(JT):
                    jsl = slice(jt * P, (jt + 1) * P)
                    vp = g.psum.tile([P, H], F32, tag="b128", bufs=2)
                    nc.tensor.matmul(vp[:], hn[:, jsl], wt(f"vw{l}"), start=True, stop=False)
                    nc.tensor.matmul(vp[:], hn[:, jsl], wt(f"vw{l}~lo"), start=False, stop=True)
                    vaug = ap_.tile([P, 33 * NH], BF16, tag=f"vaug{jt}", name=f"vaug{jt}")
                    nc.scalar.activation(
                        vaug[:].rearrange("p (h x) -> p h x", x=33)[:, :, 0:HD],
                        vp[:].rearrange("p (h x) -> p h x", x=HD), AF.Identity)
                    nc.scalar.activation(
                        vaug[:].rearrange("p (h x) -> p h x", x=33)[:, :, HD:HD + 1],
                        g.ones_b[:, 0:NH].rearrange("p (h x) -> p h x", x=1), AF.Identity)
                    vaugs.append(vaug)
                o_raw = ap_.tile([P, NG], BF16, tag="o_raw")
                rbp = g.psum.tile([P, NG], F32, tag="b512", bufs=2)
                for hh in range(NH):
                    avh = g.psum.tile([33, NG], F32, tag="av", bufs=2, name=f"avh{l}_{gr}_{hh}")
                    for jt in range(JT):
                        jsl = slice(jt * P, (jt + 1) * P)
                        sp = g.psum.tile([P, NG], F32, tag="b512", bufs=2)
                        nc.tensor.matmul(
                            sp[:], ksb[hh * HD:(hh + 1) * HD, jsl],
                            qsb[hh * HD:(hh + 1) * HD, :],
                            tile_position=(hh * HD, 0), start=True, stop=True)
                        eb = ap_.tile([P, NG], BF16, tag="eb")
                        nc.scalar.activation(eb[:], sp[:], AF.Exp, scale=1.0 / math.sqrt(HD))
                        nc.tensor.matmul(avh[:], vaugs[jt][:, hh * 33:(hh + 1) * 33], eb[:],
                                         start=(jt == 0), stop=(jt == JT - 1))
                    nc.vector.tensor_copy(out=o_raw[hh * HD:(hh + 1) * HD, :], in_=avh[0:HD, :])
                    recpf = ap_.tile([1, NG], F32, tag="recpf")
                    nc.vector.reciprocal(recpf[:], avh[HD:HD + 1, :])
                    recpb = ap_.tile([1, NG], BF16, tag="recpb")
                    nc.vector.tensor_copy(out=recpb[:], in_=recpf[:])
                    nc.tensor.matmul(rbp[hh * HD:(hh + 1) * HD, :],
                                     g.ones_b[0:1, 0:HD], recpb[:],
                                     tile_position=(0, hh * HD),
                                     start=True, stop=True)
                o_in = ap_.tile([P, NG], BF16, tag="o_in")
                nc.vector.tensor_tensor(out=o_in[:], in0=o_raw[:], in1=rbp[:], op=ALU.mult)
                op_ = g.psum.tile([P, NG], F32, tag="b512", bufs=2)
                mm2(op_[:], f"ow{l}", o_in[:])
                ob = ap_.tile([P, NG], F32, tag="ob")
                nc.scalar.activation(ob[:], op_[:], AF.Identity, bias=vc(f"ob_{l}"))
                nc.vector.tensor_tensor(out=h_sb[:, gsl], in0=h_sb[:, gsl], in1=ob[:], op=ALU.add)

            if f"dbg_hattn{l}" in dbg:
                nc.sync.dma_start(dbg[f"dbg_hattn{l}"][:], h_sb[:])

        # ---- readout ----
        mean4 = g.tmp.tile([P, GB], BF16, tag="mean4")
        max4 = g.tmp.tile([P, GB], BF16, tag="max4")
        for gr in range(GB):
            gsl = slice(gr * NG, (gr + 1) * NG)
            sm = g.tmp.tile([P, 1], F32, tag="sm")
            nc.vector.tensor_reduce(out=sm[:], in_=h_sb[:, gsl], axis=mybir.AxisListType.X,
                                    op=ALU.add)
            nc.scalar.activation(mean4[:, gr:gr + 1], sm[:], AF.Identity, scale=1.0 / NG)
            # NOTE: this environment's jax lowers segment_max to scatter-add,
            # so the reference's "x_max" is actually the segment SUM.
            nc.vector.tensor_copy(out=max4[:, gr:gr + 1], in_=sm[:])

        # add branch
        aaf = g.tmp.tile([9, GB], BF16, tag="aaf")
        nc.sync.dma_start(aaf[:], ins["augaf"])
        ap1 = g.psum.tile([P, GB], F32, tag="b128", bufs=2)
        nc.tensor.matmul(ap1[:], w9a_sb[:], aaf[:], start=True, stop=False)
        nc.tensor.matmul(ap1[:], w9al_sb[:], aaf[:], start=False, stop=True)
        at1 = g.tmp.tile([P, GB], BF16, tag="at1")
        nc.scalar.activation(at1[:], ap1[:], AF.Relu, bias=vc("ambeta"))
        ap2 = g.psum.tile([P, GB], F32, tag="b128", bufs=2)
        mm2(ap2[:], "amw2", at1[:])
        addt = g.tmp.tile([P, GB], BF16, tag="addt")
        nc.scalar.activation(addt[:], ap2[:], AF.Identity, bias=vc("amb2"))

        # comb = [mean, max, mean, add]; t1 = comb @ o_w1  (two 128-col halves)
        combs = [mean4, max4, mean4, addt]
        t1o = []
        for mc in range(2):
            tps = g.psum.tile([P, GB], F32, tag="b128", bufs=2)
            for kc in range(4):
                nc.tensor.matmul(tps[:], wt(f"ow1_{kc}_{mc}"), combs[kc][:],
                                 start=(kc == 0), stop=False)
                nc.tensor.matmul(tps[:], wt(f"ow1_{kc}_{mc}~lo"), combs[kc][:],
                                 start=False, stop=(kc == 3))
            tb = g.tmp.tile([P, GB], BF16, tag=f"t1o{mc}")
            nc.scalar.activation(tb[:], tps[:], AF.Identity, bias=vc(f"ob1{'ab'[mc]}"))
            t1o.append(tb)
        t1n = [g.tmp.tile([P, GB], BF16, tag=f"t1n{i}", name=f"t1n{i}") for i in range(2)]
        _ln_fm(g, [t1o[0][:], t1o[1][:]], GB, 2 * H, [t1n[0][:], t1n[1][:]],
               gain=[vc("og1a"), vc("og1b")], gainK=[vc("og1aK"), vc("og1bK")],
               beta=[vc("obeta1a"), vc("obeta1b")], relu=True)

        t2ps = g.psum.tile([P, GB], F32, tag="b128", bufs=2)
        for kc in range(2):
            nc.tensor.matmul(t2ps[:], wt(f"ow2_{kc}"), t1n[kc][:],
                             start=(kc == 0), stop=False)
            nc.tensor.matmul(t2ps[:], wt(f"ow2_{kc}~lo"), t1n[kc][:],
                             start=False, stop=(kc == 1))
        t2b = g.tmp.tile([P, GB], BF16, tag="t2b")
        nc.scalar.activation(t2b[:], t2ps[:], AF.Identity, bias=vc("ob2"))
        t2n = g.tmp.tile([P, GB], BF16, tag="t2n")
        _ln_fm(g, [t2b[:]], GB, H, [t2n[:]], gain=[vc("og2")], gainK=[vc("og2K")],
               beta=[vc("obeta2")], relu=True)

        t3ps = g.psum.tile([64, GB], F32, tag="b128", bufs=2)
        nc.tensor.matmul(t3ps[:], ow3_sb[:], t2n[:], start=True, stop=False)
        nc.tensor.matmul(t3ps[:], ow3l_sb[:], t2n[:], start=False, stop=True)
        t3b = g.tmp.tile([64, GB], BF16, tag="t3b")
        nc.scalar.activation(t3b[:], t3ps[:], AF.Relu, bias=vc("ob3")[0:64, :])

        if dbg:
            for nm, t in (("mean4", mean4), ("max4", max4), ("addt", addt),
                          ("t1o0", t1o[0]), ("t1o1", t1o[1]),
                          ("t1n0", t1n[0]), ("t1n1", t1n[1]),
                          ("t2b", t2b), ("t2n", t2n), ("t3b", t3b)):
                nc.sync.dma_start(dbg[f"dbg_{nm}"][:], t[:])

        t4ps = g.psum.tile([1, GB], F32, tag="b128", bufs=2)
        nc.tensor.matmul(t4ps[:], ow4_sb[:], t3b[:], start=True, stop=False)
        nc.tensor.matmul(t4ps[:], ow4l_sb[:], t3b[:], start=False, stop=True)
        outsb = g.tmp.tile([1, GB], F32, tag="outsb")
        nc.scalar.activation(outsb[:], t4ps[:], AF.Identity, bias=vc("ob4")[0:1, :])
        nc.sync.dma_start(out_dram[:], outsb[:])

        stack.close()

    nc.compile()
    return nc


# ----------------------------------------------------------------------------
# Entry point
# ----------------------------------------------------------------------------

def kernel(**inputs):
    global LAST_RESULT
    D = DIMS_FULL
    per_core, meta = preprocess(inputs, D)
    nc = build_program(D, meta, D["C"])
    core_ids = list(range(D["C"]))
    trace = bool(int(os.environ.get("BASS_KERNEL_TRACE", "0")))
    res = run_bass_kernel_spmd(nc, per_core, core_ids, trace=trace)
    LAST_RESULT = res
    outs = [np.asarray(res.results[c]["out"], np.float32).reshape(-1) for c in core_ids]
    return np.concatenate(outs).reshape(D["B"] // D["C"] * D["C"], 1).astype(np.float32)



# revision 25
# speedup vs baseline: 52.1836x; 52.1836x over previous
"""HEAGNN Trainium2 kernel: 4-layer edge-gated GNN + per-graph attention.

Sharding: graphs/nodes/edges-by-dst split across 8 cores. Per layer each core
computes node projections for its 2048-node block, AllGathers the row-major
[m | gj] gather table, gathers per-edge rows with dma_gather (window-batched),
accumulates the gate pre-activation in PSUM via PE matmuls (ea@gwe + identity
adds of gathered gi/gj), applies a batched sigmoid + message multiply, and
scatter-adds messages via host-precomputed one-hot matmuls. Host pre-folds all
LayerNorm gains/means that feed linear layers and builds the sorted/padded
edge schedule.

NOTE: this environment's jax lowers segment_max to scatter-add, so the
reference's "x_max" readout branch is actually the segment SUM. The kernel
matches that behavior.
"""

import math
import os

import ml_dtypes
import numpy as np

import concourse.bacc as bacc
import concourse.bass as bass
import concourse.mybir as mybir
import concourse.tile as tile
from concourse.bass_utils import run_bass_kernel_spmd

F32 = mybir.dt.float32
BF16 = mybir.dt.bfloat16
I16 = mybir.dt.int16
F8 = mybir.dt.float8e4
AF = mybir.ActivationFunctionType
ALU = mybir.AluOpType

P = 128
EPS = 1e-5

DIMS_FULL = dict(NN=16384, E=262144, B=32, H=128, L=4, NH=4, C=8)

LAST_RESULT = None


# ----------------------------------------------------------------------------
# Host-side math helpers
# ----------------------------------------------------------------------------

def _bf(x):
    return np.ascontiguousarray(np.asarray(x, np.float32).astype(ml_dtypes.bfloat16))


def _f32(x):
    return np.ascontiguousarray(np.asarray(x, np.float32))


def _ln_fold_pre(Wm, b, g):
    """Fold LN that FOLLOWS z=u@Wm+b.  Returns (Wfull [(in+1),H], rstd_fn).
    device computes act(aug @ Wfull + beta) with aug=[u*rstd, rstd]."""
    Wm = np.asarray(Wm, np.float64)
    b = np.asarray(b, np.float64)
    g = np.asarray(g, np.float64)
    H = Wm.shape[1]
    Wc = Wm - Wm.mean(axis=1, keepdims=True)
    bc = b - b.mean()
    A = (Wc @ Wc.T) / H
    cvec = (Wc @ bc) / H
    dconst = float((bc * bc).mean())

    def rstd_fn(u):
        u = np.asarray(u, np.float64)
        q = np.einsum("ni,ij,nj->n", u, A, u) + 2.0 * (u @ cvec) + dconst
        return 1.0 / np.sqrt(q + EPS)

    Wg = Wc * g[None, :]
    bg = bc * g
    Wfull = np.concatenate([Wg, bg[None, :]], axis=0)  # [(in+1), H]
    return Wfull, rstd_fn


def _aug(u, rstd):
    """[n,k],[n] -> [k+1, n] feature-major augmented input."""
    a = np.concatenate([u * rstd[:, None], rstd[:, None]], axis=1)
    return np.ascontiguousarray(a.T)


def _wrap16(idx):
    """Wrap an index list (len % 16 == 0) into dma_gather layout [16, n/16]."""
    return np.ascontiguousarray(idx.reshape(-1, 16).T)


# ----------------------------------------------------------------------------
# Host preprocessing: shard + sort + pad edges, fold weights
# ----------------------------------------------------------------------------

def preprocess(inputs, D):
    NN, E, B, H, L, NH, C = (D[k] for k in ("NN", "E", "B", "H", "L", "NH", "C"))
    NB = NN // C          # nodes per core
    GB = B // C           # graphs per core
    NG = NN // B          # nodes per graph
    W = NB // P           # 128-node windows per core
    assert NG == 512

    src = np.asarray(inputs["edge_index"][0], np.int64)
    dst = np.asarray(inputs["edge_index"][1], np.int64)
    ea = _f32(inputs["edge_attr"])

    order = np.argsort(dst, kind="stable")
    src_s, dst_s, ea_s = src[order], dst[order], ea[order]

    win_of = dst_s // P
    counts = np.bincount(win_of, minlength=NN // P)
    T = int(math.ceil(counts.max() / P))
    T = max(T, 1)
    NT = W * T
    EPAD = NT * P

    src_pad = np.zeros((C, EPAD), np.int64)
    dstloc_pad = np.zeros((C, EPAD), np.int64)
    dwin_pad = np.full((C, EPAD), -1, np.int64)  # sentinel: no one-hot
    ea_pad = np.zeros((C, EPAD, ea.shape[1]), np.float32)

    wstart = np.concatenate([[0], np.cumsum(counts)])
    for c in range(C):
        for w in range(W):
            gw = c * W + w
            lo, hi = wstart[gw], wstart[gw + 1]
            n = hi - lo
            off = w * T * P
            src_pad[c, off:off + n] = src_s[lo:hi]
            dstloc_pad[c, off:off + n] = dst_s[lo:hi] - c * NB
            dwin_pad[c, off:off + n] = dst_s[lo:hi] - c * NB - w * P
            ea_pad[c, off:off + n] = ea_s[lo:hi]

    # one-hot scatter matrices [e, j] and their transposes [j, e]
    selblob = np.zeros((C, P, NT * P), np.float32)
    selTblob = np.zeros((C, P, NT * P), np.float32)
    eye = np.eye(P, dtype=np.float32)
    for c in range(C):
        dw = dwin_pad[c].reshape(NT, P)  # [tile, edge-in-tile]
        for t in range(NT):
            # sel[e, j] = 1 if dst-offset-in-window == j (0 for padding)
            onehot = np.zeros((P, P), np.float32)
            valid = dw[t] >= 0
            onehot[valid] = eye[dw[t][valid]]
            selblob[c, :, t * P:(t + 1) * P] = onehot
            selTblob[c, :, t * P:(t + 1) * P] = onehot.T

    # LN folds -------------------------------------------------------------
    W6x, rstd_x = _ln_fold_pre(inputs["ne_w"], inputs["ne_b"], inputs["ne_g"])
    W4e, rstd_e = _ln_fold_pre(inputs["ee_w"], inputs["ee_b"], inputs["ee_g"])
    W9a, rstd_a = _ln_fold_pre(inputs["am_w1"], inputs["am_b1"], inputs["am_g"])

    x = _f32(inputs["x"])
    augx = np.stack([
        _aug(x[c * NB:(c + 1) * NB], rstd_x(x[c * NB:(c + 1) * NB])) for c in range(C)
    ])  # [C, 6, NB]
    auge = np.stack([
        _aug(ea_pad[c], rstd_e(ea_pad[c])) for c in range(C)
    ])  # [C, 4, EPAD]
    af = _f32(inputs["additional_features"])
    augaf = np.stack([
        _aug(af[c * GB:(c + 1) * GB], rstd_a(af[c * GB:(c + 1) * GB])) for c in range(C)
    ])  # [C, 9, GB]

    # wrapped int16 gather indices, per window, tiled to 128 partitions
    def wrapped(idx2d):  # [C, EPAD] -> [C, 128, NT*8]
        outs = []
        for c in range(C):
            cols = [
                _wrap16(idx2d[c, w * T * P:(w + 1) * T * P].astype(np.int16))
                for w in range(W)
            ]
            m = np.concatenate(cols, axis=1)  # [16, NT*8]
            outs.append(np.tile(m, (8, 1)))
        return np.stack(outs)

    srcw = wrapped(src_pad)
    dstw = wrapped(dstloc_pad)

    # attention folds per layer --------------------------------------------
    at_g = _f32(inputs["at_g"])
    at_b = _f32(inputs["at_beta"])
    qwf, kwf, vwf, owf = [], [], [], []
    qbf, kbf, obf = [], [], []
    for l in range(L):
        qw = _f32(inputs["qw"][l]); kw = _f32(inputs["kw"][l])
        vw = _f32(inputs["vw"][l]); ow = _f32(inputs["ow_"][l])
        qwf.append(at_g[l][:, None] * qw)
        kwf.append(at_g[l][:, None] * kw)
        vwf.append(at_g[l][:, None] * vw)
        owf.append(ow)
        qbf.append(at_b[l] @ qw + _f32(inputs["qb"][l]))
        kbf.append(at_b[l] @ kw + _f32(inputs["kb"][l]))
        vb_ = at_b[l] @ vw + _f32(inputs["vb"][l])
        obf.append(vb_ @ ow + _f32(inputs["ob_"][l]))

    # weight blob (bf16 [128, 128] tiles)
    HB = H
    blob = []
    blob_idx = {}

    def addw(name, w):
        w = _f32(w)
        assert w.shape == (HB, HB), (name, w.shape)
        blob_idx[name] = len(blob)
        blob.append(w)

    def addw2(name, w):
        w = _f32(w)
        hi = _bf(w).astype(np.float32)
        addw(name, hi)
        addw(name + "~lo", w - hi)

    gw_all = _f32(inputs["gw"])  # [L, 3H, H]
    for l in range(L):
        addw(f"gwe{l}", gw_all[l][:H])
    for l in range(L):
        addw(f"gwi{l}", gw_all[l][H:2 * H])
        addw(f"gwj{l}", gw_all[l][2 * H:3 * H])
        addw2(f"w1_{l}", inputs["nm_w1"][l])
        addw2(f"w2_{l}", inputs["nm_w2"][l])
        addw(f"qw{l}", qwf[l])
        addw(f"kw{l}", kwf[l])
        addw2(f"vw{l}", vwf[l])
        addw2(f"ow{l}", owf[l])
    ow1 = _f32(inputs["o_w1"])  # [4H, 2H]
    for kc in range(4):
        for mc in range(2):
            addw2(f"ow1_{kc}_{mc}", ow1[kc * H:(kc + 1) * H, mc * H:(mc + 1) * H])
    ow2 = _f32(inputs["o_w2"])  # [2H, H]
    for kc in range(2):
        addw2(f"ow2_{kc}", ow2[kc * H:(kc + 1) * H])
    addw2("amw2", inputs["am_w2"])
    wblob = _bf(np.concatenate(blob, axis=1))  # [128, 128*nw]

    # bias/vector blob f32 [128, nv]
    vecs = []
    vec_idx = {}

    def addv(name, v, n=None):
        v = _f32(v).reshape(-1)
        col = np.zeros((P,), np.float32)
        col[: v.shape[0]] = v
        vec_idx[name] = len(vecs)
        vecs.append(col)

    for l in range(L):
        addv(f"b1_{l}", inputs["nm_b1"][l])
        addv(f"g_{l}", inputs["nm_g"][l])
        addv(f"gK_{l}", _f32(inputs["nm_g"][l]) / H)
        addv(f"beta_{l}", inputs["nm_beta"][l])
        addv(f"b2_{l}", inputs["nm_b2"][l])
        addv(f"gb_{l}", inputs["gb"][l])
        addv(f"qb_{l}", qbf[l])
        addv(f"kb_{l}", kbf[l])
        addv(f"ob_{l}", obf[l])
    addv("nebeta", inputs["ne_beta"])
    addv("eebeta", inputs["ee_beta"])
    addv("ambeta", inputs["am_beta"])
    addv("amb2", inputs["am_b2"])
    og1 = _f32(inputs["o_g1"]); ob1 = _f32(inputs["o_beta1"]); obi1 = _f32(inputs["o_b1"])
    addv("og1a", og1[:H]); addv("og1b", og1[H:])
    addv("og1aK", og1[:H] / (2 * H)); addv("og1bK", og1[H:] / (2 * H))
    addv("obeta1a", ob1[:H]); addv("obeta1b", ob1[H:])
    addv("ob1a", obi1[:H]); addv("ob1b", obi1[H:])
    addv("og2", inputs["o_g2"]); addv("og2K", _f32(inputs["o_g2"]) / H)
    addv("obeta2", inputs["o_beta2"]); addv("ob2", inputs["o_b2"])
    addv("ob3", inputs["o_b3"])
    addv("ob4", inputs["o_b4"])
    addv("eps", np.full((P,), EPS, np.float32))
    vblob = _f32(np.stack(vecs, axis=1))  # [128, nv]

    # small-K weights
    def hilo(w):
        w = _f32(w)
        hi = _bf(w)
        return hi, _bf(w - hi.astype(np.float32))

    w6x_h, w6x_l = hilo(W6x)
    w4e_h, w4e_l = hilo(W4e)
    w9a_h, w9a_l = hilo(W9a)
    ow3_h, ow3_l = hilo(inputs["o_w3"])
    ow4_h, ow4_l = hilo(inputs["o_w4"])
    smalls = dict(
        w6x=w6x_h, w6x_lo=w6x_l,
        w4e=w4e_h, w4e_lo=w4e_l,
        w9a=w9a_h, w9a_lo=w9a_l,
        ow3=ow3_h, ow3_lo=ow3_l,
        ow4=ow4_h, ow4_lo=ow4_l,
    )

    identy = _bf(np.eye(P, dtype=np.float32))
    ones_b = _bf(np.ones((P, P), np.float32))

    meta = dict(NB=NB, GB=GB, NG=NG, W=W, T=T, NT=NT, EPAD=EPAD,
                blob_idx=blob_idx, vec_idx=vec_idx, nw=len(blob), nv=len(vecs))

    per_core = []
    for c in range(C):
        per_core.append(dict(
            augx=_bf(augx[c]), auge=_bf(auge[c]), augaf=_bf(augaf[c]),
            srcw=srcw[c].astype(np.int16),
            selblob=_bf(selblob[c]), selTblob=_bf(selTblob[c]),
            wblob=wblob, vblob=vblob,
            identy=identy, ones_b=ones_b,
            **smalls,
        ))
    return per_core, meta


# ----------------------------------------------------------------------------
# Device program
# ----------------------------------------------------------------------------

class Ctx:
    pass


def _ln_fm(g, zbs, n, Kfeat, outs, gain=None, gainK=None, beta=None, relu=False):
    """Feature-major LayerNorm over partition dim (Kfeat = 128 or 256).
    zbs: list of bf16 SBUF APs [128, n] (feature tiles).  outs: same-shaped
    bf16 out APs.  gain/gainK/beta: [128,1] f32 APs per tile (or None)."""
    nc, ps, tp = g.nc, g.psum, g.tmp
    s_ps = ps.tile([P, n], F32, tag="b512", bufs=2)
    for i, z in enumerate(zbs):
        nc.tensor.matmul(s_ps[:], g.ones_b[:], z, start=(i == 0), stop=(i == len(zbs) - 1))
    s_sb = tp.tile([P, n], F32, tag="ln_ssb", bufs=2)
    nc.vector.tensor_copy(out=s_sb[:], in_=s_ps[:])
    q_ps = ps.tile([P, n], F32, tag="b512", bufs=2)
    for i, z in enumerate(zbs):
        z2 = tp.tile([P, n], BF16, tag="ln_z2", bufs=2)
        nc.vector.tensor_tensor(out=z2[:], in0=z, in1=z, op=ALU.mult)
        nc.tensor.matmul(q_ps[:], g.ones_b[:], z2[:], start=(i == 0), stop=(i == len(zbs) - 1))
    t1 = tp.tile([P, n], F32, tag="ln_x", bufs=2)
    nc.vector.scalar_tensor_tensor(out=t1[:], in0=s_sb[:], scalar=1.0 / Kfeat,
                                   in1=s_sb[:], op0=ALU.mult, op1=ALU.mult)
    d = tp.tile([P, n], F32, tag="ln_y", bufs=2)
    nc.vector.tensor_tensor(out=d[:], in0=q_ps[:], in1=t1[:], op=ALU.subtract)
    sd = tp.tile([P, n], F32, tag="ln_z", bufs=2)
    nc.scalar.activation(sd[:], d[:], AF.Sqrt, bias=g.eps, scale=1.0 / Kfeat)
    rstd = tp.tile([P, n], F32, tag="ln_rs", bufs=2)
    nc.vector.reciprocal(rstd[:], sd[:])
    for i, (z, out) in enumerate(zip(zbs, outs)):
        a1 = tp.tile([P, n], F32, tag="ln_x", bufs=2)
        nc.vector.scalar_tensor_tensor(out=a1[:], in0=z, scalar=(gain[i] if gain else 1.0),
                                       in1=rstd[:], op0=ALU.mult, op1=ALU.mult)
        a2 = tp.tile([P, n], F32, tag="ln_y", bufs=2)
        nc.vector.scalar_tensor_tensor(out=a2[:], in0=s_sb[:],
                                       scalar=(gainK[i] if gainK else 1.0 / Kfeat),
                                       in1=rstd[:], op0=ALU.mult, op1=ALU.mult)
        a3 = tp.tile([P, n], F32, tag="ln_z", bufs=2)
        nc.vector.tensor_tensor(out=a3[:], in0=a1[:], in1=a2[:], op=ALU.subtract)
        nc.scalar.activation(out, a3[:], AF.Relu if relu else AF.Identity,
                             bias=(beta[i] if beta else 0.0))


def build_program(D, meta, num_cores):
    NN, E, B, H, L, NH, C = (D[k] for k in ("NN", "E", "B", "H", "L", "NH", "C"))
    NB, GB, NG, W, T, NT, EPAD = (meta[k] for k in ("NB", "GB", "NG", "W", "T", "NT", "EPAD"))
    HD = H // NH
    CH = min(512, NB)  # node-phase chunk width
    assert NB % CH == 0

    nc = bacc.Bacc("TRN2", target_bir_lowering=False, debug=False,
                   num_devices=num_cores)

    def din(name, shape, dt):
        return nc.dram_tensor(name, list(shape), dt, kind="ExternalInput").ap()

    ins = dict(
        augx=din("augx", (6, NB), BF16),
        auge=din("auge", (4, EPAD), BF16),
        augaf=din("augaf", (9, GB), BF16),
        srcw=din("srcw", (P, NT * 8), I16),
        selblob=din("selblob", (P, NT * P), BF16),
        selTblob=din("selTblob", (P, NT * P), BF16),
        wblob=din("wblob", (P, P * meta["nw"]), BF16),
        vblob=din("vblob", (P, meta["nv"]), F32),
        identy=din("identy", (P, P), BF16),
        ones_b=din("ones_b", (P, P), BF16),
        w6x=din("w6x", (6, H), BF16),
        w4e=din("w4e", (4, H), BF16),
        w9a=din("w9a", (9, H), BF16),
        ow3=din("ow3", (H, 64), BF16),
        ow4=din("ow4", (64, 1), BF16),
        w6x_lo=din("w6x_lo", (6, H), BF16),
        w4e_lo=din("w4e_lo", (4, H), BF16),
        w9a_lo=din("w9a_lo", (9, H), BF16),
        ow3_lo=din("ow3_lo", (H, 64), BF16),
        ow4_lo=din("ow4_lo", (64, 1), BF16),
    )
    out_dram = nc.dram_tensor("out", [1, GB], F32, kind="ExternalOutput").ap()

    dbg = {}
    if bool(int(os.environ.get("BASS_DEBUG_DUMPS", "0"))):
        def dout(name, shape, dt=F32):
            dbg[name] = nc.dram_tensor(name, list(shape), dt, kind="ExternalOutput").ap()
        dout("dbg_h0", (P, NB))
        for l in range(L):
            dout(f"dbg_m{l}", (P, NB), BF16)
            dout(f"dbg_hconv{l}", (P, NB))
            dout(f"dbg_hattn{l}", (P, NB))

    ea_pm = nc.dram_tensor("ea_pm", [P, EPAD], BF16).ap()
    mg_in = nc.dram_tensor("mg_in", [NB, 2 * H], F8).ap()
    mg_tab = nc.dram_tensor("mg_tab", [NN, 2 * H], F8, addr_space="Shared").ap()

    bi, vi = meta["blob_idx"], meta["vec_idx"]

    with tile.TileContext(nc) as tc:
        g = Ctx()
        g.nc = nc
        import contextlib
        stack = contextlib.ExitStack()
        cpool = stack.enter_context(tc.tile_pool(name="const", bufs=1))
        g.psum = stack.enter_context(tc.tile_pool(name="ps", bufs=1, space="PSUM"))
        g.tmp = stack.enter_context(tc.tile_pool(name="tmp", bufs=3))
        pers = stack.enter_context(tc.tile_pool(name="pers", bufs=1))
        ep = stack.enter_context(tc.tile_pool(name="ep", bufs=2))
        et = stack.enter_context(tc.tile_pool(name="et", bufs=3))
        ap_ = stack.enter_context(tc.tile_pool(name="ap", bufs=2))

        # ---- load constants ----
        wblob_sb = cpool.tile([P, P * meta["nw"]], BF16)
        nc.sync.dma_start(wblob_sb[:], ins["wblob"])
        vblob_sb = cpool.tile([P, meta["nv"]], F32)
        nc.sync.dma_start(vblob_sb[:], ins["vblob"])

        def wt(name):
            i = bi[name]
            return wblob_sb[:, i * P:(i + 1) * P]

        def vc(name):
            i = vi[name]
            return vblob_sb[:, i:i + 1]

        g.eps = vc("eps")
        g.ones_b = cpool.tile([P, P], BF16)
        nc.sync.dma_start(g.ones_b[:], ins["ones_b"])
        ident_sb = cpool.tile([P, P], BF16)
        nc.sync.dma_start(ident_sb[:], ins["identy"])
        identf_sb = cpool.tile([P, P], F32)
        nc.vector.tensor_copy(out=identf_sb[:], in_=ident_sb[:])
        w6x_sb = cpool.tile([6, H], BF16)
        nc.sync.dma_start(w6x_sb[:], ins["w6x"])
        w4e_sb = cpool.tile([4, H], BF16)
        nc.sync.dma_start(w4e_sb[:], ins["w4e"])
        w9a_sb = cpool.tile([9, H], BF16)
        nc.sync.dma_start(w9a_sb[:], ins["w9a"])
        ow3_sb = cpool.tile([H, 64], BF16)
        nc.sync.dma_start(ow3_sb[:], ins["ow3"])
        ow4_sb = cpool.tile([64, 1], BF16)
        nc.sync.dma_start(ow4_sb[:], ins["ow4"])
        w6xl_sb = cpool.tile([6, H], BF16)
        nc.sync.dma_start(w6xl_sb[:], ins["w6x_lo"])
        w4el_sb = cpool.tile([4, H], BF16)
        nc.sync.dma_start(w4el_sb[:], ins["w4e_lo"])
        w9al_sb = cpool.tile([9, H], BF16)
        nc.sync.dma_start(w9al_sb[:], ins["w9a_lo"])
        ow3l_sb = cpool.tile([H, 64], BF16)
        nc.sync.dma_start(ow3l_sb[:], ins["ow3_lo"])
        ow4l_sb = cpool.tile([64, 1], BF16)
        nc.sync.dma_start(ow4l_sb[:], ins["ow4_lo"])

        def mm2(psum_ap, name, rhs):
            nc.tensor.matmul(psum_ap, wt(name), rhs, start=True, stop=False)
            nc.tensor.matmul(psum_ap, wt(name + "~lo"), rhs, start=False, stop=True)

        srcw_sb = pers.tile([P, NT * 8], I16)
        nc.sync.dma_start(srcw_sb[:], ins["srcw"])

        h_sb = pers.tile([P, NB], F32)      # node features, feature-major
        m_sb = pers.tile([P, NB], BF16)     # node_mlp output (per layer)
        gj_sb = pers.tile([P, NB], BF16)
        gi_sb = pers.tile([P, NB], BF16)
        gi_nm = pers.tile([P, W * H], BF16)  # gi node-major per window

        # ---- embed: h0 ----
        augx_sb = cpool.tile([6, NB], BF16)
        nc.sync.dma_start(augx_sb[:], ins["augx"])
        for k in range(NB // CH):
            sl = slice(k * CH, (k + 1) * CH)
            zp = g.psum.tile([P, CH], F32, tag="b512", bufs=2)
            nc.tensor.matmul(zp[:], w6x_sb[:], augx_sb[:, sl], start=True, stop=False)
            nc.tensor.matmul(zp[:], w6xl_sb[:], augx_sb[:, sl], start=False, stop=True)
            nc.scalar.activation(h_sb[:, sl], zp[:], AF.Relu, bias=vc("nebeta"))

        # ---- embed: ea -> ea_pm (feature-major, bf16) ----
        ECH = 512
        for off in range(0, EPAD, ECH):
            n = min(ECH, EPAD - off)
            sl = slice(off, off + n)
            ae = g.tmp.tile([4, ECH], BF16, tag="auge")
            nc.sync.dma_start(ae[:, :n], ins["auge"][:, sl])
            zp = g.psum.tile([P, ECH], F32, tag="b512", bufs=2)
            nc.tensor.matmul(zp[:, :n], w4e_sb[:], ae[:, :n], start=True, stop=False)
            nc.tensor.matmul(zp[:, :n], w4el_sb[:], ae[:, :n], start=False, stop=True)
            eat = g.tmp.tile([P, ECH], BF16, tag="eat")
            nc.scalar.activation(eat[:, :n], zp[:, :n], AF.Relu, bias=vc("eebeta"))
            nc.sync.dma_start(ea_pm[:, sl], eat[:, :n])

        if "dbg_h0" in dbg:
            nc.sync.dma_start(dbg["dbg_h0"][:], h_sb[:])

        # ---- layers ----
        for l in range(L):
            # node phase: m, gj, gi on own block
            for k in range(NB // CH):
                sl = slice(k * CH, (k + 1) * CH)
                hb = g.tmp.tile([P, CH], BF16, tag="hb")
                nc.vector.tensor_copy(out=hb[:], in_=h_sb[:, sl])
                zp = g.psum.tile([P, CH], F32, tag="b512", bufs=2)
                mm2(zp[:], f"w1_{l}", hb[:])
                zb = g.tmp.tile([P, CH], BF16, tag="zb")
                nc.scalar.activation(zb[:], zp[:], AF.Identity, bias=vc(f"b1_{l}"))
                tb = g.tmp.tile([P, CH], BF16, tag="tb")
                _ln_fm(g, [zb[:]], CH, H, [tb[:]], gain=[vc(f"g_{l}")],
                       gainK=[vc(f"gK_{l}")], beta=[vc(f"beta_{l}")], relu=True)
                mp = g.psum.tile([P, CH], F32, tag="b512", bufs=2)
                mm2(mp[:], f"w2_{l}", tb[:])
                nc.scalar.activation(m_sb[:, sl], mp[:], AF.Identity, bias=vc(f"b2_{l}"))
                gp = g.psum.tile([P, CH], F32, tag="b512", bufs=2)
                nc.tensor.matmul(gp[:], wt(f"gwj{l}"), hb[:], start=True, stop=True)
                nc.scalar.activation(gj_sb[:, sl], gp[:], AF.Identity)
                gp2 = g.psum.tile([P, CH], F32, tag="b512", bufs=2)
                nc.tensor.matmul(gp2[:], wt(f"gwi{l}"), hb[:], start=True, stop=True)
                nc.scalar.activation(gi_sb[:, sl], gp2[:], AF.Identity, bias=vc(f"gb_{l}"))

            # transpose m|gj -> mg_in rows; gi -> gi_dram rows
            for i in range(NB // P):
                tsl = slice(i * P, (i + 1) * P)
                stg = g.tmp.tile([P, 2 * H], F8, tag="mgstg")
                tp1 = g.psum.tile([P, P], BF16, tag="b128", bufs=2)
                nc.tensor.transpose(tp1[:], m_sb[:, tsl], ident_sb[:])
                nc.vector.tensor_copy(out=stg[:, 0:H], in_=tp1[:])
                tp2 = g.psum.tile([P, P], BF16, tag="b128", bufs=2)
                nc.tensor.transpose(tp2[:], gj_sb[:, tsl], ident_sb[:])
                nc.vector.tensor_copy(out=stg[:, H:2 * H], in_=tp2[:])
                nc.sync.dma_start(mg_in[tsl, :], stg[:])
                tp3 = g.psum.tile([P, P], BF16, tag="b128", bufs=2)
                nc.tensor.transpose(tp3[:], gi_sb[:, tsl], ident_sb[:])
                nc.vector.tensor_copy(out=gi_nm[:, i * H:(i + 1) * H], in_=tp3[:])

            # AllGather mg table
            nc.gpsimd.collective_compute(
                "AllGather", ALU.bypass,
                replica_groups=[list(range(num_cores))],
                ins=[mg_in[:]], outs=[mg_tab[:]],
            )

            # edge phase: per-window gather batches, sel streamed from DRAM
            for w in range(W):
                mgb = ep.tile([P, T * 2 * H], F8, tag="mgb")
                GCT = 4  # tiles per dma_gather call (SWDGE ring holds ~1024 descs)
                for t0 in range(0, T, GCT):
                    t1 = min(t0 + GCT, T)
                    nct = t1 - t0
                    i0 = w * T * 8 + t0 * 8
                    nc.gpsimd.dma_gather(
                        mgb[:, t0 * 2 * H:t1 * 2 * H].rearrange(
                            "p (t e) -> p t e", e=2 * H),
                        mg_tab[:], srcw_sb[:, i0:i0 + nct * 8],
                        nct * P, nct * P, 2 * H)
                eab = ep.tile([P, T * P], BF16, tag="eab")
                nc.sync.dma_start(eab[:], ea_pm[:, w * T * P:(w + 1) * T * P])
                selw = ep.tile([P, T * P], BF16, tag="selw")
                nc.sync.dma_start(selw[:],
                                  ins["selblob"][:, w * T * P:(w + 1) * T * P])
                selwT = ep.tile([P, T * P], BF16, tag="selwT")
                nc.sync.dma_start(selwT[:],
                                  ins["selTblob"][:, w * T * P:(w + 1) * T * P])

                agg = g.psum.tile([P, P], F32, tag="agg", bufs=1)
                first = True
                for t0 in range(0, T, 4):
                    t1 = min(t0 + 4, T)
                    gn = t1 - t0
                    pre = g.psum.tile([P, 4 * P], F32, tag="b512", bufs=2)
                    for i, t in enumerate(range(t0, t1)):
                        psl = slice(i * P, (i + 1) * P)
                        nc.tensor.matmul(pre[:, psl], eab[:, t * P:(t + 1) * P],
                                         wt(f"gwe{l}"), start=True, stop=False)
                        nc.tensor.matmul(pre[:, psl], selwT[:, t * P:(t + 1) * P],
                                         gi_nm[:, w * H:(w + 1) * H],
                                         start=False, stop=False)
                        nc.tensor.matmul(pre[:, psl], ident_sb[:],
                                         mgb[:, t * 2 * H + H:(t + 1) * 2 * H],
                                         start=False, stop=True)
                    gate = et.tile([P, 4 * P], BF16, tag="gate")
                    nc.scalar.activation(gate[:, :gn * P], pre[:, :gn * P],
                                         AF.Sigmoid)
                    msg = et.tile([P, 4 * P], BF16, tag="msg")
                    nc.vector.tensor_tensor(
                        out=msg[:, :gn * P], in0=gate[:, :gn * P],
                        in1=mgb[:].rearrange("p (t e) -> p t e", e=2 * H)[
                            :, t0:t1, 0:H],
                        op=ALU.mult)
                    for i, t in enumerate(range(t0, t1)):
                        nc.tensor.matmul(
                            agg[:], msg[:, i * P:(i + 1) * P],
                            selw[:, t * P:(t + 1) * P],
                            start=first, stop=False)
                        first = False
                # fold m into agg: agg[f, j] += m[f, j]
                hsl = slice(w * P, (w + 1) * P)
                nc.tensor.matmul(agg[:], ident_sb[:], m_sb[:, hsl],
                                 start=False, stop=True)
                nc.vector.tensor_tensor(out=h_sb[:, hsl], in0=h_sb[:, hsl],
                                        in1=agg[:], op=ALU.add)

            if f"dbg_hconv{l}" in dbg:
                nc.sync.dma_start(dbg[f"dbg_m{l}"][:], m_sb[:])
                nc.sync.dma_start(dbg[f"dbg_hconv{l}"][:], h_sb[:])

            # attention: pass 1 = LN + q/k/v for all graphs (Sqrt table),
            # pass 2 = scores/softmax/out (Exp table)
            qsbs, ksbs, vaugs_all = [], [], []
            for gr in range(GB):
                gsl = slice(gr * NG, (gr + 1) * NG)
                hgb = ap_.tile([P, NG], BF16, tag="hgb")
                nc.vector.tensor_copy(out=hgb[:], in_=h_sb[:, gsl])
                hn = ap_.tile([P, NG], BF16, tag="hn")
                _ln_fm(g, [hgb[:]], NG, H, [hn[:]])
                qp = g.psum.tile([P, NG], F32, tag="b512", bufs=2)
                nc.tensor.matmul(qp[:], wt(f"qw{l}"), hn[:], start=True, stop=True)
                qsb = ap_.tile([P, NG], BF16, tag=f"qsb{gr}", name=f"qsb{l}_{gr}")
                nc.scalar.activation(qsb[:], qp[:], AF.Identity, bias=vc(f"qb_{l}"))
                kp = g.psum.tile([P, NG], F32, tag="b512", bufs=2)
                nc.tensor.matmul(kp[:], wt(f"kw{l}"), hn[:], start=True, stop=True)
                ksb = ap_.tile([P, NG], BF16, tag=f"ksb{gr}", name=f"ksb{l}_{gr}")
                nc.scalar.activation(ksb[:], kp[:], AF.Identity, bias=vc(f"kb_{l}"))
                JT = NG // P
                vaugs = []
                for jt in range(JT):
                    jsl = slice(jt * P, (jt + 1) * P)
                    vp = g.psum.tile([P, H], F32, tag="b128", bufs=2)
                    nc.tensor.matmul(vp[:], hn[:, jsl], wt(f"vw{l}"), start=True, stop=False)
                    nc.tensor.matmul(vp[:], hn[:, jsl], wt(f"vw{l}~lo"), start=False, stop=True)
                    vaug = ap_.tile([P, 33 * NH], BF16, tag=f"vaug{gr}_{jt}",
                                    name=f"vaug{l}_{gr}_{jt}")
                    nc.scalar.activation(
                        vaug[:].rearrange("p (h x) -> p h x", x=33)[:, :, 0:HD],
                        vp[:].rearrange("p (h x) -> p h x", x=HD), AF.Identity)
                    nc.scalar.activation(
                        vaug[:].rearrange("p (h x) -> p h x", x=33)[:, :, HD:HD + 1],
                        g.ones_b[:, 0:NH].rearrange("p (h x) -> p h x", x=1), AF.Identity)
                    vaugs.append(vaug)
                qsbs.append(qsb); ksbs.append(ksb); vaugs_all.append(vaugs)

            for gr in range(GB):
                gsl = slice(gr * NG, (gr + 1) * NG)
                qsb, ksb, vaugs = qsbs[gr], ksbs[gr], vaugs_all[gr]
                JT = NG // P
                o_raw = ap_.tile([P, NG], BF16, tag="o_raw")
                rbp = g.psum.tile([P, NG], F32, tag="b512", bufs=2)
                for hh in range(NH):
                    avh = g.psum.tile([33, NG], F32, tag="av", bufs=2,
                                      name=f"avh{l}_{gr}_{hh}")
                    for jt in range(JT):
                        jsl = slice(jt * P, (jt + 1) * P)
                        sp = g.psum.tile([P, NG], F32, tag="b512", bufs=2)
                        nc.tensor.matmul(
                            sp[:], ksb[hh * HD:(hh + 1) * HD, jsl],
                            qsb[hh * HD:(hh + 1) * HD, :],
                            tile_position=(hh * HD, 0), start=True, stop=True)
                        eb = ap_.tile([P, NG], BF16, tag="eb")
                        nc.scalar.activation(eb[:], sp[:], AF.Exp,
                                             scale=1.0 / math.sqrt(HD))
                        nc.tensor.matmul(avh[:], vaugs[jt][:, hh * 33:(hh + 1) * 33],
                                         eb[:], start=(jt == 0), stop=(jt == JT - 1))
                    nc.vector.tensor_copy(out=o_raw[hh * HD:(hh + 1) * HD, :],
                                          in_=avh[0:HD, :])
                    recpf = ap_.tile([1, NG], F32, tag="recpf")
                    nc.vector.reciprocal(recpf[:], avh[HD:HD + 1, :])
                    recpb = ap_.tile([1, NG], BF16, tag="recpb")
                    nc.vector.tensor_copy(out=recpb[:], in_=recpf[:])
                    nc.tensor.matmul(rbp[hh * HD:(hh + 1) * HD, :],
                                     g.ones_b[0:1, 0:HD], recpb[:],
                                     tile_position=(0, hh * HD),
                                     start=True, stop=True)
                o_in = ap_.tile([P, NG], BF16, tag="o_in")
                nc.vector.tensor_tensor(out=o_in[:], in0=o_raw[:], in1=rbp[:], op=ALU.mult)
                op_ = g.psum.tile([P, NG], F32, tag="b512", bufs=2)
                mm2(op_[:], f"ow{l}", o_in[:])
                ob = ap_.tile([P, NG], F32, tag="ob")
                nc.scalar.activation(ob[:], op_[:], AF.Identity, bias=vc(f"ob_{l}"))
                nc.vector.tensor_tensor(out=h_sb[:, gsl], in0=h_sb[:, gsl], in1=ob[:],
                                        op=ALU.add)

            if f"dbg_hattn{l}" in dbg:
                nc.sync.dma_start(dbg[f"dbg_hattn{l}"][:], h_sb[:])

        # ---- readout ----
        mean4 = g.tmp.tile([P, GB], BF16, tag="mean4")
        max4 = g.tmp.tile([P, GB], BF16, tag="max4")
        for gr in range(GB):
            gsl = slice(gr * NG, (gr + 1) * NG)
            sm = g.tmp.tile([P, 1], F32, tag="sm")
            nc.vector.tensor_reduce(out=sm[:], in_=h_sb[:, gsl], axis=mybir.AxisListType.X,
                                    op=ALU.add)
            nc.scalar.activation(mean4[:, gr:gr + 1], sm[:], AF.Identity, scale=1.0 / NG)
            # this environment's jax lowers segment_max to scatter-add, so the
            # reference's "x_max" branch is actually the segment SUM.
            nc.vector.tensor_copy(out=max4[:, gr:gr + 1], in_=sm[:])

        # add branch
        aaf = g.tmp.tile([9, GB], BF16, tag="aaf")
        nc.sync.dma_start(aaf[:], ins["augaf"])
        ap1 = g.psum.tile([P, GB], F32, tag="b128", bufs=2)
        nc.tensor.matmul(ap1[:], w9a_sb[:], aaf[:], start=True, stop=False)
        nc.tensor.matmul(ap1[:], w9al_sb[:], aaf[:], start=False, stop=True)
        at1 = g.tmp.tile([P, GB], BF16, tag="at1")
        nc.scalar.activation(at1[:], ap1[:], AF.Relu, bias=vc("ambeta"))
        ap2 = g.psum.tile([P, GB], F32, tag="b128", bufs=2)
        mm2(ap2[:], "amw2", at1[:])
        addt = g.tmp.tile([P, GB], BF16, tag="addt")
        nc.scalar.activation(addt[:], ap2[:], AF.Identity, bias=vc("amb2"))

        # comb = [mean, max, mean, add]; t1 = comb @ o_w1  (two 128-col halves)
        combs = [mean4, max4, mean4, addt]
        t1o = []
        for mc in range(2):
            tps = g.psum.tile([P, GB], F32, tag="b128", bufs=2)
            for kc in range(4):
                nc.tensor.matmul(tps[:], wt(f"ow1_{kc}_{mc}"), combs[kc][:],
                                 start=(kc == 0), stop=False)
                nc.tensor.matmul(tps[:], wt(f"ow1_{kc}_{mc}~lo"), combs[kc][:],
                                 start=False, stop=(kc == 3))
            tb = g.tmp.tile([P, GB], BF16, tag=f"t1o{mc}")
            nc.scalar.activation(tb[:], tps[:], AF.Identity, bias=vc(f"ob1{'ab'[mc]}"))
            t1o.append(tb)
        t1n = [g.tmp.tile([P, GB], BF16, tag=f"t1n{i}", name=f"t1n{i}") for i in range(2)]
        _ln_fm(g, [t1o[0][:], t1o[1][:]], GB, 2 * H, [t1n[0][:], t1n[1][:]],
               gain=[vc("og1a"), vc("og1b")], gainK=[vc("og1aK"), vc("og1bK")],
               beta=[vc("obeta1a"), vc("obeta1b")], relu=True)

        t2ps = g.psum.tile([P, GB], F32, tag="b128", bufs=2)
        for kc in range(2):
            nc.tensor.matmul(t2ps[:], wt(f"ow2_{kc}"), t1n[kc][:],
                             start=(kc == 0), stop=False)
            nc.tensor.matmul(t2ps[:], wt(f"ow2_{kc}~lo"), t1n[kc][:],
                             start=False, stop=(kc == 1))
        t2b = g.tmp.tile([P, GB], BF16, tag="t2b")
        nc.scalar.activation(t2b[:], t2ps[:], AF.Identity, bias=vc("ob2"))
        t2n = g.tmp.tile([P, GB], BF16, tag="t2n")
        _ln_fm(g, [t2b[:]], GB, H, [t2n[:]], gain=[vc("og2")], gainK=[vc("og2K")],
               beta=[vc("obeta2")], relu=True)

        t3ps = g.psum.tile([64, GB], F32, tag="b128", bufs=2)
        nc.tensor.matmul(t3ps[:], ow3_sb[:], t2n[:], start=True, stop=False)
        nc.tensor.matmul(t3ps[:], ow3l_sb[:], t2n[:], start=False, stop=True)
        t3b = g.tmp.tile([64, GB], BF16, tag="t3b")
        nc.scalar.activation(t3b[:], t3ps[:], AF.Relu, bias=vc("ob3")[0:64, :])

        t4ps = g.psum.tile([1, GB], F32, tag="b128", bufs=2)
        nc.tensor.matmul(t4ps[:], ow4_sb[:], t3b[:], start=True, stop=False)
        nc.tensor.matmul(t4ps[:], ow4l_sb[:], t3b[:], start=False, stop=True)
        outsb = g.tmp.tile([1, GB], F32, tag="outsb")
        nc.scalar.activation(outsb[:], t4ps[:], AF.Identity, bias=vc("ob4")[0:1, :])
        nc.sync.dma_start(out_dram[:], outsb[:])

        stack.close()

    nc.compile()
    return nc


# ----------------------------------------------------------------------------
# Entry point
# ----------------------------------------------------------------------------

SIM_TIME_NS = None


def kernel(**inputs):
    global LAST_RESULT, SIM_TIME_NS
    D = DIMS_FULL
    per_core, meta = preprocess(inputs, D)
    nc = build_program(D, meta, D["C"])
    if bool(int(os.environ.get("BASS_SIM_TIME", "0"))):
        try:
            from concourse.bass_interp import CoreSim
            sim = CoreSim(nc, trace=False, no_exec=True, publish_trace=False)
            sim.simulate()
            SIM_TIME_NS = int(sim.time)
        except Exception:
            SIM_TIME_NS = None
    core_ids = list(range(D["C"]))
    trace = bool(int(os.environ.get("BASS_KERNEL_TRACE", "0")))
    res = run_bass_kernel_spmd(nc, per_core, core_ids, trace=trace)
    LAST_RESULT = res
    outs = [np.asarray(res.results[c]["out"], np.float32).reshape(-1) for c in core_ids]
    return np.concatenate(outs).reshape(D["B"] // D["C"] * D["C"], 1).astype(np.float32)
